# revision 54
# baseline (speedup 1.0000x reference)
"""BitPackedLinear Trainium2 kernel (8-core SPMD, token-sharded, fp8 DR).

y = x @ W.T + bias, W = unpack_bits(packed_weight) in {-1,+1}, shapes:
  x [2, 2048, 4096] f32, packed_weight [4096, 512] u8, bias [4096] f32.

Sharding: data-parallel over tokens (4096 tokens -> 512/core). Each core
computes y_c = x_c @ W.T + bias for its token shard against the full
weight; the host just concatenates shards.

Device algorithm per core (main GEMM in fp8e4 DoubleRow, 0.5 cyc/row --
2x the bf16 row rate, contracting two 128-i blocks per instruction):
  - x is split as x = hi + lo with hi = fp8(bf16(x)), lo = fp8(bf16(x)
    - hi); W is unpacked to {0, 2.0} and the result corrected as
    y = (hi+lo)@(2B) - rowsum(hi+lo) + bias.  The lo term is kept only
    for i-tile pairs j < NJ_LO (24 of 32 i-tiles): measured rel err
    1.34e-2 vs the 2e-2 gate, for 4 fewer DR matmuls per psum group.
  - x chunks arrive bf16 tt-major (SWDGE cast-DMA; each tt's kt=0 chunk
    goes HWDGE f32 + ACT cast -- routing the first i-block through the
    SWDGE cast corrupts on hw), bit-sliced layout i = 1024kt + 8k + b.
  - PE transposes 8 bit-planes of one chunk into one [128, 1024] PSUM
    bank; one ACT copy casts it to the fp8 hi plane and one DVE
    subtract writes the lo plane: xT_hi/xT_lo [128 i, 32 it, 512 tok],
    tokens contiguous (the dual-fp8 Ldweights ISA check requires a
    unit-stride stationary m dim; the moving ifmap tolerates stride 2).
  - byte path: slabs 0-1 of byteT via pk DMA + ACT u8->bf16 + PE
    transposes (the DRAM-bounce path has ~6us/DMA queue latency, too
    slow for early slabs); slabs 2-7 via a flattened u8->u16 SWDGE cast
    to DRAM (gated behind the 12th x chunk by a tiny WAW write so its
    8.7us transfer never blocks the x stream) + one XBAR transpose-DMA
    per kt into per-kt byteT tiles (separate tiles so the XBARs don't
    serialize on tile-granular WAW tracking).
  - W unpack is 1 DVE op/tile: wt16 = (u16(byte) << (14-b)) & 0x4000
    puts fp8 {0, 2.0} in the HIGH byte; DR matmuls read odd bytes
    ([p, 2, 512] pair-stride 1024, n-stride 2 -- validated on hw).
    Slab 0/1 unpack ops are woven between the lo casts in the DVE
    stream; slabs 2-7 unpack per-slab in the main loop (wt double
    buffered).
  - Main DR matmuls per (o-slab, token-tile), j ascending, hi then lo:
    psum += plane[:, 2j:2j+2, t].T (x) wt8[:, 2j:2j+2, :].  Slab 0's
    four groups emit their DR streams first, then the s_col DR matmuls
    (rowsum(hi+lo) vs an all-ones stationary, consistent with the kept
    lo pairs), then the rank-2 finishers -- the in-order PE queue never
    parks on s2.
  - bias + (-s_col) enter each psum via ONE rank-2 f32r matmul
    (k=0: s_col x -1, k=1: ones x bias) closing the accumulation group;
    epilogue is an ACT psum->SBUF copy + DMA.  The f32r operand rows
    live at partition 0 (engine APs must start on a quarter boundary);
    partition-1 rows are DMA-filled.
  - 24 identity warm-up transposes ramp the PE p-state during DMA fill.
"""
import sys

sys.path.insert(0, "/opt/trn_rl_repo")
from contextlib import ExitStack

import numpy as np

import concourse.tile as tile
from concourse import bacc, mybir
from concourse.bass import ts
from concourse.bass_utils import run_bass_kernel_spmd
from concourse.masks import make_identity

F32 = mybir.dt.float32
F32R = mybir.dt.float32r
BF16 = mybir.dt.bfloat16
F8 = mybir.dt.float8e4
U8 = mybir.dt.uint8
U16 = mybir.dt.uint16
P = 128
DR = mybir.MatmulPerfMode.DoubleRow
ACT_COPY = mybir.ActivationFunctionType.Copy

N_CORES = 8
B_DIM, S_DIM, I_DIM, O_DIM = 2, 2048, 4096, 4096
T_FULL = B_DIM * S_DIM          # 4096 tokens
T_SHARD = T_FULL // N_CORES     # 512 tokens per core
OUT_NAME = "y"
OUT_SHAPE = (T_SHARD, O_DIM)


def build(T=T_SHARD, I=I_DIM, O=O_DIM, O_SLAB=512, n_cores=N_CORES,
          byte_mode="dmat"):
    assert I % 1024 == 0 and T % P == 0 and O % P == 0 and O % O_SLAB == 0
    KT = I // 1024          # 128-byte groups along i (4)
    IT = KT * 8             # bit-sliced i-tiles (32)
    NJ = IT // 2            # DR i-tile pairs (16)
    TT = T // P             # token tiles (4)
    K = I // 8              # packed bytes per weight row (512)
    NSLAB = O // O_SLAB
    OSL_T = O_SLAB // P
    NJ_LO = 12              # lo-term kept for j < NJ_LO (i-tiles 0-23);
                            # dropping kt3's lo saves 4 DR matmuls/group at
                            # ~1.2e-2 rel err (gate is 2e-2)
    SHB, MASK = 14, 0x4000  # unpack: fp8 {0,2.0} pattern in the HIGH byte

    nc = bacc.Bacc("TRN2", target_bir_lowering=False, debug=False,
                   num_devices=n_cores)
    x_d = nc.dram_tensor("x", [T, I], F32, kind="ExternalInput").ap()
    pw_d = nc.dram_tensor("pw", [O, K], U8, kind="ExternalInput").ap()
    bias_d = nc.dram_tensor("bias", [O], F32, kind="ExternalInput").ap()
    y_d = nc.dram_tensor(OUT_NAME, [T, O], F32, kind="ExternalOutput").ap()
    pw16_d = nc.dram_tensor("pw16", [O, K], U16).ap()

    with tile.TileContext(nc) as tc:
        with ExitStack() as ctx:
            const = ctx.enter_context(tc.tile_pool(name="const", bufs=1))
            persist = ctx.enter_context(tc.tile_pool(name="persist", bufs=1))
            stage = ctx.enter_context(tc.tile_pool(name="stage", bufs=1))

            ident_bf = const.tile([P, P], BF16)
            make_identity(nc, ident_bf[:])
            ones2 = const.tile([P, 2, P], F8)
            nc.vector.memset(ones2[:], 1.0)

            # rank-2 psum-init operands; engine-written rows sit at
            # partition 0 (engine APs must start on a quarter boundary),
            # partition-1 rows are DMA-filled.
            # ps = s2[0,t]*br2[0,o] + s2[1,t]*br2[1,o]
            #    = s_col[t]*(-1)    + 1*bias[o]
            s2 = const.tile([2, T], F32R)    # row0 = s_col, row1 = ones
            br2 = const.tile([2, O], F32R)   # row0 = -ones, row1 = bias

            byteTs = [persist.tile([P, O], U16, name=f"byteT{k}") for k in range(KT)]
            xT_hi = persist.tile([P, IT, T], F8)
            xT_lo = persist.tile([P, IT, T], F8)

            xn_pool = ctx.enter_context(tc.tile_pool(name="xnat", bufs=10))
            x32_pool = ctx.enter_context(tc.tile_pool(name="x32", bufs=2))
            pk_pool = ctx.enter_context(tc.tile_pool(name="pk", bufs=2))
            pkbf_pool = ctx.enter_context(tc.tile_pool(name="pkbf", bufs=2))
            wt_pool = ctx.enter_context(tc.tile_pool(name="wt", bufs=2))
            y_pool = ctx.enter_context(tc.tile_pool(name="ysb", bufs=3))
            ps_tr = ctx.enter_context(
                tc.tile_pool(name="ps_tr", bufs=3, space="PSUM")
            )
            ps_mm = ctx.enter_context(
                tc.tile_pool(name="ps_mm", bufs=5, space="PSUM")
            )


            def emit_bias_stage():
                nc.sync.dma_start(
                    br2[1:2, :],
                    bias_d.bitcast(F32R).rearrange("(b o) -> b o", b=1),
                )
                cst = stage.tile([1, T], F32)
                nc.vector.memset(cst[:], -1.0)
                nc.vector.tensor_copy(
                    out=br2[0:1, :],
                    in_=cst[:, :1].broadcast_to([1, O]),
                )
                ones_row = stage.tile([1, T], F32R)
                nc.vector.memset(cst[:], 1.0)
                nc.vector.tensor_copy(out=ones_row[:], in_=cst[:])
                return ones_row

            ones_row = emit_bias_stage()

            def byte_slab(sl):
                """Fill byteT[:, :, sl*O_SLAB:(sl+1)*O_SLAB] via the PE
                (latency-free vs the XBAR queue for early slabs)."""
                pw_ap = pw_d.rearrange("(ot p) k -> p ot k", p=P)
                pk = pk_pool.tile([P, OSL_T, K], U8)
                nc.sync.dma_start(pk[:], pw_ap[:, ts(sl, OSL_T), :])
                for otl in range(OSL_T):
                    ot = sl * OSL_T + otl
                    pkbf = pkbf_pool.tile([P, K], BF16)
                    nc.scalar.activation(out=pkbf[:], in_=pk[:, otl, :],
                                         func=ACT_COPY)
                    ps = ps_tr.tile([P, 8, P], BF16, tag="tr_ps")
                    for kt in range(KT):
                        nc.tensor.transpose(
                            ps[:, kt, :], pkbf[:, ts(kt, P)], ident_bf[:]
                        )
                    for kt in range(KT):
                        dst = byteTs[kt][:, ts(ot, P)]
                        if (otl + kt) % 2 == 0:
                            nc.vector.tensor_copy(out=dst, in_=ps[:, kt, :])
                        else:
                            nc.scalar.activation(out=dst, in_=ps[:, kt, :],
                                                 func=ACT_COPY)

            # PE warm-up: back-to-back identity transposes ramp the
            # p-state to full clock while the first DMAs land.
            ps_warm = ps_tr.tile([P, 8, P], BF16, tag="tr_ps")
            for i in range(24):
                nc.tensor.transpose(
                    ps_warm[:, i % 8, :], ident_bf[:], ident_bf[:]
                )

            def unpack_part(wt, sl, kts):
                for kt in kts:
                    for b in range(8):
                        it = kt * 8 + b
                        nc.vector.tensor_scalar(
                            out=wt[:, it, :],
                            in0=byteTs[kt][:, ts(sl, O_SLAB)],
                            scalar1=SHB - b, scalar2=MASK,
                            op0=mybir.AluOpType.logical_shift_left,
                            op1=mybir.AluOpType.bitwise_and,
                        )

            def unpack_slab(sl):
                wt = wt_pool.tile([P, IT, O_SLAB], U16, tag="wt")
                unpack_part(wt, sl, range(KT))
                return wt

            # ---- byte path: slabs 0-1 via the PE (the XBAR/SWDGE
            # bounce has ~6us per-DMA queue latency; early slabs cannot
            # wait for it) ----
            byte_slab(0)
            wt0 = wt_pool.tile([P, IT, O_SLAB], U16, tag="wt")
            unpack_part(wt0, 0, [0, 1])
            byte_slab(1)
            wt1 = wt_pool.tile([P, IT, O_SLAB], U16, tag="wt")

            # ---- x chunks: ONE SWDGE cast-DMA per token-tile
            # ([128, 4096] f32->bf16 is still 128 descriptors -- 4x the
            # data per Q7 descriptor-gen, so all of x lands ~4x sooner)
            xns = {}
            for tt in range(TT):
                for kt in range(KT):
                    src_ap = x_d[ts(tt, P), ts(kt, 1024)].rearrange(
                        "p (k b) -> p k b", b=8
                    )
                    xn = xn_pool.tile([P, P, 8], BF16, tag="xn16")
                    if kt == 0:
                        x32 = x32_pool.tile([P, P, 8], F32)
                        nc.sync.dma_start(x32[:], src_ap)
                        nc.scalar.activation(out=xn[:], in_=x32[:], func=ACT_COPY)
                    else:
                        nc.gpsimd.dma_start(xn[:], src_ap)
                    xns[kt, tt] = xn

            # s2 ones row (DMA here so it doesn't head-block the sync
            # queue while waiting for the DVE-built constant)
            nc.sync.dma_start(s2[1:2, :], ones_row[:])

            # pw16 bounce + XBARs for slabs 2-7.  The tiny WAW write
            # below (sourced from the last x chunk) gates the 8.7us cast
            # transfer behind the x stream on the DMA engines; slab 2
            # isn't consumed until ~50us so there is plenty of slack.
            nc.sync.dma_start(
                pw16_d[2 * O_SLAB:2 * O_SLAB + 1, :1],
                xns[KT - 1, TT - 2][:1, :1, 0].bitcast(U16),
            )
            nc.gpsimd.dma_start(
                out=pw16_d[2 * O_SLAB:, :].rearrange("a b -> (a b)"),
                in_=pw_d[2 * O_SLAB:, :].rearrange("a b -> (a b)"),
            )
            for kt in range(KT):
                nc.sync.dma_start_transpose(
                    byteTs[kt][:, 2 * O_SLAB:],
                    pw16_d[2 * O_SLAB:, ts(kt, P)],
                )

            # ---- transposes tt-major, 8 bit-planes of one chunk batched
            # into one [128, 1024] PSUM bank; hi (ACT) / lo (DVE) casts
            # read PSUM directly ----
            for tt in range(TT):
                for kt in range(KT):
                    ps = ps_tr.tile([P, 8, P], BF16, tag="tr_ps")
                    for b in range(8):
                        nc.tensor.transpose(
                            ps[:, b, :], xns[kt, tt][:, :, b], ident_bf[:]
                        )
                    hi = xT_hi[:, ts(kt, 8), ts(tt, P)]
                    lo = xT_lo[:, ts(kt, 8), ts(tt, P)]
                    nc.scalar.activation(out=hi, in_=ps[:], func=ACT_COPY)
                    if kt * 8 < 2 * NJ_LO:
                        nc.vector.tensor_tensor(
                            out=lo, in0=ps[:], in1=hi,
                            op=mybir.AluOpType.subtract
                        )
                # weave the remaining slab-0/1 unpack ops between the lo
                # casts so neither the DR stream nor the planes starve
                if tt == 0:
                    unpack_part(wt0, 0, [2, 3])
                elif tt == 1:
                    unpack_part(wt1, 1, [0, 1])
                elif tt == 2:
                    unpack_part(wt1, 1, [2, 3])

            # ---- main o-slab loop.  Slab 0 is special: its 4 groups'
            # DR streams are emitted first (they only need wt0 and their
            # own token-tile's planes), then the s_col DR matmuls (which
            # need ALL planes), then the rank-2 finishers -- so the
            # in-order PE queue never parks on s2. ----
            def emit_group_drs(ps, wt8, tsub):
                for j in range(NJ):
                    for plane in (xT_hi, xT_lo):
                        if plane is xT_lo and j >= NJ_LO:
                            continue
                        nc.tensor.matmul(
                            ps[:],
                            plane[:, 2 * j:2 * j + 2, ts(tsub, P)],
                            wt8[:, 2 * j:2 * j + 2, :, 1],
                            start=(j == 0 and plane is xT_hi),
                            stop=False,
                            perf_mode=DR,
                        )

            def emit_finish(ps, sl, tsub):
                # bias - s_col enters last; closes the accumulation group
                nc.tensor.matmul(
                    ps[:], s2[:, ts(tsub, P)], br2[:, ts(sl, O_SLAB)],
                    start=False, stop=True,
                )
                y_sb = y_pool.tile([P, O_SLAB], F32)
                nc.scalar.activation(out=y_sb[:], in_=ps[:], func=ACT_COPY)
                nc.sync.dma_start(
                    y_d[ts(tsub, P), ts(sl, O_SLAB)], y_sb[:]
                )

            wt0_8 = wt0[:].bitcast(F8).rearrange(
                "p it (n two) -> p it n two", two=2
            )
            ps0 = []
            for tsub in range(TT):
                ps = ps_mm.tile([P, O_SLAB], F32)
                emit_group_drs(ps, wt0_8, tsub)
                ps0.append(ps)

            # s_col = rowsum(hi+lo) via DR matmuls vs an all-ones
            # stationary; then the s2 row and slab-0 finishers
            ps_s = ps_tr.tile([P, T], F32, tag="tr_ps")
            for pi, plane in enumerate((xT_hi, xT_lo)):
                nj = NJ if pi == 0 else NJ_LO
                for j in range(nj):
                    nc.tensor.matmul(
                        ps_s[:], ones2[:], plane[:, 2 * j:2 * j + 2, :],
                        start=(pi == 0 and j == 0),
                        stop=(pi == 1 and j == nj - 1),
                        perf_mode=DR,
                    )
            nc.vector.tensor_copy(out=s2[0:1, :], in_=ps_s[0:1, :])
            for tsub in range(TT):
                emit_finish(ps0[tsub], 0, tsub)

            for sl in range(1, NSLAB):
                wt = wt1 if sl == 1 else unpack_slab(sl)
                wt8 = wt[:].bitcast(F8).rearrange(
                    "p it (n two) -> p it n two", two=2
                )
                for tsub in range(TT):
                    ps = ps_mm.tile([P, O_SLAB], F32)
                    emit_group_drs(ps, wt8, tsub)
                    emit_finish(ps, sl, tsub)

    nc.compile()
    return nc


_NC = None


def _get_nc():
    global _NC
    if _NC is None:
        _NC = build()
    return _NC


def run(x, packed_weight, bias, trace=False):
    x = np.ascontiguousarray(np.asarray(x, dtype=np.float32))
    pw = np.ascontiguousarray(np.asarray(packed_weight, dtype=np.uint8))
    bias = np.ascontiguousarray(np.asarray(bias, dtype=np.float32))
    assert x.shape == (B_DIM, S_DIM, I_DIM)
    assert pw.shape == (O_DIM, I_DIM // 8)
    assert bias.shape == (O_DIM,)

    nc = _get_nc()
    xs = x.reshape(T_FULL, I_DIM)
    in_maps = [
        {
            "x": np.ascontiguousarray(xs[c * T_SHARD:(c + 1) * T_SHARD]),
            "pw": pw,
            "bias": bias,
        }
        for c in range(N_CORES)
    ]
    res = run_bass_kernel_spmd(nc, in_maps, list(range(N_CORES)), trace=trace)
    y = np.concatenate(
        [res.results[c][OUT_NAME] for c in range(N_CORES)], axis=0
    )
    return y.reshape(B_DIM, S_DIM, O_DIM), res


def kernel(x, packed_weight, bias):
    y, _ = run(x, packed_weight, bias, trace=False)
    return y
